# revision 5
# baseline (speedup 1.0000x reference)
"""Local2d (locally-connected conv, unshared weights) Trainium2 kernel.

Problem: out[b,o,h,w] = sum_{i,k,l} weight[o,h,w,i,k,l] * xpad[b,i,h+k,w+l] + bias[o,h,w]
  x: [64, 64, 32, 32] f32, weight: [128, 32, 32, 64, 3, 3] f32, bias: [128, 32, 32] f32
  out: [64, 128, 32, 32] f32

Strategy: shard the 32 output rows h across 8 cores (4 rows each). Each output
location (h,w) is an independent GEMM: [o=128] x [ikl=576] @ [ikl=576] x [b=64].
Host pre-transposes weight to [h, ikl, w, o] and pre-extracts patches to
[h, ikl, w, b] (both fp16) so the contraction dim lands on SBUF partitions with
large contiguous DMAs and no on-chip transposes. Per location: 5 PSUM-accumulated
matmuls over ikl chunks (4x128 + 64), then a DVE bias-add copy into an output
tile [o, w, b] written back once per row. Output is reassembled/transposed on
host. Matmul inputs in fp16 (exact products, fp32 PSUM accumulation): rel err
vs the fp32 reference ~5e-4.
"""

import os
import numpy as np

B, C_IN, C_OUT, KS, H, W = 64, 64, 128, 3, 32, 32
H_OUT, W_OUT = 32, 32
N_CORES = 8
H_PER = H_OUT // N_CORES  # 4
IKL = C_IN * KS * KS  # 576
CHUNKS = [(0, 128), (128, 128), (256, 128), (384, 128), (512, 64)]

_NC_CACHE = {}
_RUNNER_CACHE = {}
_LAST_IN_MAPS = None
LAST_RESULT = None


def _split_multiwaits(nc):
    """This container's walrus accepts at most ONE sync-wait per instruction.
    Hoist extra waits onto single-wait NoOps on the same engine, inserted
    immediately before (engine streams are in-order, sem waits are >=-monotonic,
    so this is semantics-preserving)."""
    import concourse.mybir as mybir

    ctr = 0
    hist = {}
    for f in nc.m.functions:
        for blk in f.blocks:
            insts = list(blk.instructions)
            changed = False
            newlist = []
            for inst in insts:
                si = inst.sync_info
                if si is not None and si.on_wait and len(si.on_wait) > 1:
                    tname = type(inst).__name__
                    hist[tname] = hist.get(tname, 0) + 1
                    waits = list(si.on_wait)
                    for wt in waits[:-1]:
                        nop = mybir.InstNoOp(name=f"splitwait-{ctr}", ins=[], outs=[])
                        ctr += 1
                        nop.engine = inst.engine
                        nop.sync_info = mybir.SyncInfo(on_wait=[wt], on_update=[])
                        newlist.append(nop)
                    inst.sync_info = mybir.SyncInfo(
                        on_wait=[waits[-1]], on_update=list(si.on_update or [])
                    )
                    changed = True
                newlist.append(inst)
            if changed:
                blk.instructions = newlist
    if os.environ.get("K_DEBUG"):
        print(f"split_multiwaits: {ctr} extra waits hoisted; by type: {hist}")
    return ctr


def _build_nc(dt_name, reps=1):
    import concourse.bass as bass
    import concourse.mybir as mybir
    import concourse.tile as tile

    dt_in = getattr(mybir.dt, dt_name)
    nc = bass.Bass()
    w_d = nc.dram_tensor("w", [H_PER, IKL, W_OUT, C_OUT], dt_in, kind="ExternalInput")
    p_d = nc.dram_tensor("p", [H_PER, IKL, W_OUT, B], dt_in, kind="ExternalInput")
    b_d = nc.dram_tensor(
        "bias", [C_OUT, H_PER, W_OUT], mybir.dt.float32, kind="ExternalInput"
    )
    o_d = nc.dram_tensor(
        "out", [C_OUT, H_PER, W_OUT, B], mybir.dt.float32, kind="ExternalOutput"
    )

    with tile.TileContext(nc) as tc:
        with (
            tc.tile_pool(name="wp", bufs=2) as wp,
            tc.tile_pool(name="pp", bufs=2) as pp,
            tc.tile_pool(name="op", bufs=2) as op,
            tc.tile_pool(name="bp", bufs=1) as bp,
            tc.tile_pool(name="psp", bufs=8, space="PSUM") as psp,
        ):
            bias_sb = bp.tile([C_OUT, H_PER, W_OUT], mybir.dt.float32, name="bias_sb")
            nc.scalar.dma_start(bias_sb[:], b_d[:])
            for rep in range(reps):
                for h in range(H_PER):
                    wts = []
                    pts = []
                    for ci, (k0, kn) in enumerate(CHUNKS):
                        wt = wp.tile(
                            [kn, W_OUT, C_OUT], dt_in, tag=f"wt{ci}",
                            name=f"wt{ci}_{rep}_{h}",
                        )
                        nc.sync.dma_start(wt[:], w_d[h, k0 : k0 + kn])
                        wts.append(wt)
                        pt = pp.tile(
                            [kn, W_OUT, B], dt_in, tag=f"pt{ci}",
                            name=f"pt{ci}_{rep}_{h}",
                        )
                        nc.scalar.dma_start(pt[:], p_d[h, k0 : k0 + kn])
                        pts.append(pt)
                    ot = op.tile(
                        [C_OUT, W_OUT, B], mybir.dt.float32, tag="ot",
                        name=f"ot_{rep}_{h}",
                    )
                    for w in range(W_OUT):
                        ps = psp.tile(
                            [C_OUT, B], mybir.dt.float32, tag="ps",
                            name=f"ps_{rep}_{h}_{w}",
                        )
                        for ci in range(len(CHUNKS)):
                            nc.tensor.matmul(
                                ps[:],
                                wts[ci][:, w, :],
                                pts[ci][:, w, :],
                                start=(ci == 0),
                                stop=(ci == len(CHUNKS) - 1),
                            )
                        nc.vector.tensor_scalar_add(
                            ot[:, w, :], ps[:], bias_sb[:, h, w : w + 1]
                        )
                    nc.gpsimd.dma_start(o_d[:, h], ot[:])

    _split_multiwaits(nc)
    return nc


def _get_nc(dt_name, reps=1):
    key = (dt_name, reps)
    if key not in _NC_CACHE:
        _NC_CACHE[key] = _build_nc(dt_name, reps)
    return _NC_CACHE[key]


def _prepare_in_maps(x, weight, bias, dt_np):
    x = np.asarray(x, dtype=np.float32)
    weight = np.asarray(weight, dtype=np.float32)
    bias = np.asarray(bias, dtype=np.float32)

    # patches P[h, i*9+k*3+l, w, b] = xpad[b, i, h+k, w+l]
    xp = np.zeros((B, C_IN, H + 2, W + 2), dtype=dt_np)
    xp[:, :, 1 : H + 1, 1 : W + 1] = x
    s = xp.strides
    v = np.lib.stride_tricks.as_strided(
        xp, (B, C_IN, KS, KS, H_OUT, W_OUT), (s[0], s[1], s[2], s[3], s[2], s[3])
    )
    P = v.transpose(4, 1, 2, 3, 5, 0).reshape(H_OUT, IKL, W_OUT, B)

    # weight -> [h, ikl, w, o]
    Wt = weight.reshape(C_OUT, H_OUT, W_OUT, IKL).transpose(1, 3, 2, 0).astype(dt_np)

    in_maps = []
    for c in range(N_CORES):
        h0 = c * H_PER
        in_maps.append(
            {
                "w": np.ascontiguousarray(Wt[h0 : h0 + H_PER]),
                "p": np.ascontiguousarray(P[h0 : h0 + H_PER]),
                "bias": np.ascontiguousarray(bias[:, h0 : h0 + H_PER, :]),
            }
        )
    return in_maps


def kernel(x, weight, bias):
    global LAST_RESULT, _LAST_IN_MAPS
    from concourse.bass_utils import run_bass_kernel_spmd

    dt_name = os.environ.get("K_DTYPE", "float16")
    dt_np = {"float16": np.float16, "float32": np.float32}[dt_name]

    in_maps = _prepare_in_maps(x, weight, bias, dt_np)
    _LAST_IN_MAPS = in_maps

    nc = _get_nc(dt_name)
    res = run_bass_kernel_spmd(nc, in_maps, list(range(N_CORES)))
    LAST_RESULT = res

    out = np.concatenate(
        [res.results[c]["out"] for c in range(N_CORES)], axis=1
    )  # [o, 32, 32, b]
    return np.ascontiguousarray(out.transpose(3, 0, 1, 2))


# ---------------------------------------------------------------------------
# Timing (NTFF profiling is unavailable in this container: antenv.axon_hooks
# missing). Measure differentially instead: jit the NEFF exec for reps=1 and
# reps=R bodies, pre-stage inputs on devices, time N pipelined executions of
# each, and report (T_R - T_1) / (N * (R - 1)).
# ---------------------------------------------------------------------------


def _make_runner(nc):
    import jax
    import concourse.mybir as mybir
    from concourse.bass2jax import (
        _bass_exec_p,
        install_neuronx_cc_hook,
        partition_id_tensor,
    )
    from jax.experimental.shard_map import shard_map
    from jax.sharding import Mesh, NamedSharding, PartitionSpec

    install_neuronx_cc_hook()

    partition_name = nc.partition_id_tensor.name if nc.partition_id_tensor else None
    in_names, out_names, out_avals, zero_outs = [], [], [], []
    for alloc in nc.m.functions[0].allocations:
        if not isinstance(alloc, mybir.MemoryLocationSet):
            continue
        name = alloc.memorylocations[0].name
        if alloc.kind == "ExternalInput":
            if name != partition_name:
                in_names.append(name)
        elif alloc.kind == "ExternalOutput":
            out_names.append(name)
            shape = tuple(alloc.tensor_shape)
            dtype = mybir.dt.np(alloc.dtype)
            out_avals.append(jax.core.ShapedArray(shape, dtype))
            zero_outs.append(np.zeros(shape, dtype))
    n_params = len(in_names)
    all_names = in_names + out_names
    if partition_name is not None:
        all_names = all_names + [partition_name]

    def _body(*args):
        operands = list(args)
        if partition_name is not None:
            operands.append(partition_id_tensor())
        outs = _bass_exec_p.bind(
            *operands,
            out_avals=tuple(out_avals),
            in_names=tuple(all_names),
            out_names=tuple(out_names),
            lowering_input_output_aliases=(),
            sim_require_finite=True,
            sim_require_nnan=True,
            nc=nc,
        )
        return tuple(outs)

    devices = jax.devices()[:N_CORES]
    mesh = Mesh(np.asarray(devices), ("core",))
    nspecs = n_params + len(out_names)
    fn = jax.jit(
        shard_map(
            _body,
            mesh=mesh,
            in_specs=(PartitionSpec("core"),) * nspecs,
            out_specs=(PartitionSpec("core"),) * len(out_names),
            check_rep=False,
        ),
        keep_unused=True,
    )
    sharding = NamedSharding(mesh, PartitionSpec("core"))
    return fn, in_names, zero_outs, sharding


def _timed_runner(dt_name, reps, in_maps):
    import jax

    key = (dt_name, reps)
    if key not in _RUNNER_CACHE:
        nc = _get_nc(dt_name, reps)
        _RUNNER_CACHE[key] = _make_runner(nc)
    fn, in_names, zero_outs, sharding = _RUNNER_CACHE[key]

    concat_in = [
        jax.device_put(
            np.concatenate([m[name] for m in in_maps], axis=0), sharding
        )
        for name in in_names
    ]
    concat_zero = [
        jax.device_put(
            np.zeros((N_CORES * z.shape[0], *z.shape[1:]), z.dtype), sharding
        )
        for z in zero_outs
    ]
    jax.block_until_ready(concat_in)

    def run_n(n):
        import time

        outs = fn(*concat_in, *concat_zero)  # warmup (compile)
        jax.block_until_ready(outs)
        outs = fn(*concat_in, *concat_zero)
        jax.block_until_ready(outs)
        t0 = time.perf_counter()
        last = None
        for _ in range(n):
            last = fn(*concat_in, *concat_zero)
        jax.block_until_ready(last)
        return time.perf_counter() - t0

    return run_n


def time_kernel_ns(n_iter=24, reps=5):
    """Differential HW time per kernel invocation, in ns."""
    assert _LAST_IN_MAPS is not None, "call kernel() first"
    dt_name = os.environ.get("K_DTYPE", "float16")
    run1 = _timed_runner(dt_name, 1, _LAST_IN_MAPS)
    runR = _timed_runner(dt_name, reps, _LAST_IN_MAPS)
    t1 = min(run1(n_iter) for _ in range(2))
    tR = min(runR(n_iter) for _ in range(2))
    per_rep = (tR - t1) / (n_iter * (reps - 1))
    if os.environ.get("K_DEBUG"):
        print(
            f"timing: T1={t1 / n_iter * 1e6:.1f} us/call, "
            f"T{reps}={tR / n_iter * 1e6:.1f} us/call, "
            f"diff/rep={per_rep * 1e6:.1f} us"
        )
    return per_rep * 1e9


# revision 11
# speedup vs baseline: 1.4746x; 1.4746x over previous
"""Local2d (locally-connected conv, unshared weights) Trainium2 kernel.

Problem: out[b,o,h,w] = sum_{i,k,l} weight[o,h,w,i,k,l] * xpad[b,i,h+k,w+l] + bias[o,h,w]
  x: [64, 64, 32, 32] f32, weight: [128, 32, 32, 64, 3, 3] f32, bias: [128, 32, 32] f32
  out: [64, 128, 32, 32] f32

Strategy: shard the 32 output rows h across 8 cores (4 rows each). Each output
location (h,w) is an independent GEMM: [o=128] x [ikl=576] @ [ikl=576] x [b=64].
Host pre-transposes weight to [h, ikl, w, o] and pre-extracts patches to
[h, ikl, w, b] (both fp16) so the contraction dim lands on SBUF partitions with
large contiguous DMAs and no on-chip transposes. Per location: 5 PSUM-accumulated
matmuls over ikl chunks (4x128 + 64), then a DVE bias-add copy into an output
tile [o, w, b] written back once per row. Output is reassembled/transposed on
host. Matmul inputs in fp16 (exact products, fp32 PSUM accumulation): rel err
vs the fp32 reference ~5e-4.
"""

import os
import numpy as np

B, C_IN, C_OUT, KS, H, W = 64, 64, 128, 3, 32, 32
H_OUT, W_OUT = 32, 32
N_CORES = 8
H_PER = H_OUT // N_CORES  # 4
IKL = C_IN * KS * KS  # 576
NCHUNK = 5
IKLP = NCHUNK * 128  # 640, ikl zero-padded so every chunk is K=128 (FWL-eligible)

_NC_CACHE = {}
_RUNNER_CACHE = {}
_LAST_IN_MAPS = None
LAST_RESULT = None


def _split_multiwaits(nc):
    """This container's walrus accepts at most ONE sync-wait per instruction.
    Hoist extra waits onto single-wait NoOps on the same engine, inserted
    immediately before (engine streams are in-order, sem waits are >=-monotonic,
    so this is semantics-preserving)."""
    import concourse.mybir as mybir

    ctr = 0
    hist = {}
    for f in nc.m.functions:
        for blk in f.blocks:
            insts = list(blk.instructions)
            changed = False
            newlist = []
            for inst in insts:
                si = inst.sync_info
                if si is not None and si.on_wait and len(si.on_wait) > 1:
                    tname = type(inst).__name__
                    hist[tname] = hist.get(tname, 0) + 1
                    waits = list(si.on_wait)
                    for wt in waits[:-1]:
                        nop = mybir.InstNoOp(name=f"splitwait-{ctr}", ins=[], outs=[])
                        ctr += 1
                        nop.engine = inst.engine
                        nop.sync_info = mybir.SyncInfo(on_wait=[wt], on_update=[])
                        newlist.append(nop)
                    inst.sync_info = mybir.SyncInfo(
                        on_wait=[waits[-1]], on_update=list(si.on_update or [])
                    )
                    changed = True
                newlist.append(inst)
            if changed:
                blk.instructions = newlist
    if os.environ.get("K_DEBUG"):
        print(f"split_multiwaits: {ctr} extra waits hoisted; by type: {hist}")
    return ctr


def _build_nc(dt_name, reps=1):
    import concourse.bass as bass
    import concourse.mybir as mybir
    import concourse.tile as tile

    dt_in = getattr(mybir.dt, dt_name)
    nc = bass.Bass()
    # ikl padded to 640 = 5*128; stored pre-chunked as [h, c, p, w, ...] so one
    # DMA per (h, tensor) lands directly in SBUF [p, c, w, ...] layout.
    w_d = nc.dram_tensor(
        "w", [H_PER, NCHUNK, 128, W_OUT, C_OUT], dt_in, kind="ExternalInput"
    )
    p_d = nc.dram_tensor(
        "p", [H_PER, NCHUNK, 128, W_OUT, B], dt_in, kind="ExternalInput"
    )
    b_d = nc.dram_tensor(
        "bias", [C_OUT, H_PER, W_OUT], mybir.dt.float32, kind="ExternalInput"
    )
    o_d = nc.dram_tensor(
        "out", [C_OUT, H_PER, W_OUT, B], mybir.dt.float32, kind="ExternalOutput"
    )

    with tile.TileContext(nc) as tc:
        with (
            tc.tile_pool(name="wp", bufs=2) as wp,
            tc.tile_pool(name="pp", bufs=2) as pp,
            tc.tile_pool(name="op", bufs=2) as op,
            tc.tile_pool(name="bp", bufs=1) as bp,
            tc.tile_pool(name="psp", bufs=8, space="PSUM") as psp,
        ):
            bias_sb = bp.tile([C_OUT, H_PER, W_OUT], mybir.dt.float32, name="bias_sb")
            nc.gpsimd.dma_start(bias_sb[:], b_d[:])
            for rep in range(reps):
                for h in range(H_PER):
                    # alternate the two HWDGE rings between the big streams
                    weng = nc.sync if h % 2 == 0 else nc.scalar
                    peng = nc.scalar if h % 2 == 0 else nc.sync
                    wt = wp.tile(
                        [128, NCHUNK, W_OUT, C_OUT], dt_in, tag="wt",
                        name=f"wt_{rep}_{h}",
                    )
                    weng.dma_start(
                        wt[:], w_d[h].rearrange("c p w o -> p c w o")
                    )
                    pt = pp.tile(
                        [128, NCHUNK, W_OUT, B], dt_in, tag="pt",
                        name=f"pt_{rep}_{h}",
                    )
                    peng.dma_start(
                        pt[:], p_d[h].rearrange("c p w b -> p c w b")
                    )
                    ot = op.tile(
                        [C_OUT, W_OUT, B], mybir.dt.float32, tag="ot",
                        name=f"ot_{rep}_{h}",
                    )
                    for w in range(W_OUT):
                        ps = psp.tile(
                            [C_OUT, B], mybir.dt.float32, tag="ps",
                            name=f"ps_{rep}_{h}_{w}",
                        )
                        for ci in range(NCHUNK):
                            nc.tensor.matmul(
                                ps[:],
                                wt[:, ci, w, :],
                                pt[:, ci, w, :],
                                start=(ci == 0),
                                stop=(ci == NCHUNK - 1),
                            )
                        nc.vector.tensor_scalar_add(
                            ot[:, w, :], ps[:], bias_sb[:, h, w : w + 1]
                        )
                    nc.gpsimd.dma_start(o_d[:, h], ot[:])

    _split_multiwaits(nc)
    return nc


def _get_nc(dt_name, reps=1):
    key = (dt_name, reps)
    if key not in _NC_CACHE:
        _NC_CACHE[key] = _build_nc(dt_name, reps)
    return _NC_CACHE[key]


def _prepare_in_maps(x, weight, bias, dt_np):
    x = np.asarray(x, dtype=np.float32)
    weight = np.asarray(weight, dtype=np.float32)
    bias = np.asarray(bias, dtype=np.float32)

    # patches P[h, i*9+k*3+l, w, b] = xpad[b, i, h+k, w+l]
    xp = np.zeros((B, C_IN, H + 2, W + 2), dtype=dt_np)
    xp[:, :, 1 : H + 1, 1 : W + 1] = x
    s = xp.strides
    v = np.lib.stride_tricks.as_strided(
        xp, (B, C_IN, KS, KS, H_OUT, W_OUT), (s[0], s[1], s[2], s[3], s[2], s[3])
    )
    P = np.zeros((H_OUT, IKLP, W_OUT, B), dtype=dt_np)
    P[:, :IKL] = v.transpose(4, 1, 2, 3, 5, 0).reshape(H_OUT, IKL, W_OUT, B)

    # weight -> [h, ikl(pad 640), w, o]
    Wt = np.zeros((H_OUT, IKLP, W_OUT, C_OUT), dtype=dt_np)
    Wt[:, :IKL] = (
        weight.reshape(C_OUT, H_OUT, W_OUT, IKL).transpose(1, 3, 2, 0)
    )

    Wt = Wt.reshape(H_OUT, NCHUNK, 128, W_OUT, C_OUT)
    P = P.reshape(H_OUT, NCHUNK, 128, W_OUT, B)

    in_maps = []
    for c in range(N_CORES):
        h0 = c * H_PER
        in_maps.append(
            {
                "w": np.ascontiguousarray(Wt[h0 : h0 + H_PER]),
                "p": np.ascontiguousarray(P[h0 : h0 + H_PER]),
                "bias": np.ascontiguousarray(bias[:, h0 : h0 + H_PER, :]),
            }
        )
    return in_maps


def kernel(x, weight, bias):
    global _LAST_IN_MAPS

    dt_name = os.environ.get("K_DTYPE", "float16")
    dt_np = {"float16": np.float16, "float32": np.float32}[dt_name]

    in_maps = _prepare_in_maps(x, weight, bias, dt_np)
    _LAST_IN_MAPS = in_maps

    fn, in_names, zero_outs, sharding = _get_runner(dt_name, 1)
    concat_in, concat_zero = _stage(
        dt_name, in_maps, in_names, zero_outs, sharding, fresh=True
    )
    outs = fn(*concat_in, *concat_zero)
    out_global = np.asarray(outs[0])  # (8*128, H_PER, 32, 64)

    out = np.concatenate(
        [out_global[c * C_OUT : (c + 1) * C_OUT] for c in range(N_CORES)], axis=1
    )  # [o, 32, 32, b]
    return np.ascontiguousarray(out.transpose(3, 0, 1, 2))


# ---------------------------------------------------------------------------
# Timing (NTFF profiling is unavailable in this container: antenv.axon_hooks
# missing). Measure differentially instead: jit the NEFF exec for reps=1 and
# reps=R bodies, pre-stage inputs on devices, time N pipelined executions of
# each, and report (T_R - T_1) / (N * (R - 1)).
# ---------------------------------------------------------------------------


def _make_runner(nc):
    import jax
    import concourse.mybir as mybir
    from concourse.bass2jax import (
        _bass_exec_p,
        install_neuronx_cc_hook,
        partition_id_tensor,
    )
    from jax.experimental.shard_map import shard_map
    from jax.sharding import Mesh, NamedSharding, PartitionSpec

    install_neuronx_cc_hook()

    partition_name = nc.partition_id_tensor.name if nc.partition_id_tensor else None
    in_names, out_names, out_avals, zero_outs = [], [], [], []
    for alloc in nc.m.functions[0].allocations:
        if not isinstance(alloc, mybir.MemoryLocationSet):
            continue
        name = alloc.memorylocations[0].name
        if alloc.kind == "ExternalInput":
            if name != partition_name:
                in_names.append(name)
        elif alloc.kind == "ExternalOutput":
            out_names.append(name)
            shape = tuple(alloc.tensor_shape)
            dtype = mybir.dt.np(alloc.dtype)
            out_avals.append(jax.core.ShapedArray(shape, dtype))
            zero_outs.append(np.zeros(shape, dtype))
    n_params = len(in_names)
    all_names = in_names + out_names
    if partition_name is not None:
        all_names = all_names + [partition_name]

    def _body(*args):
        operands = list(args)
        if partition_name is not None:
            operands.append(partition_id_tensor())
        outs = _bass_exec_p.bind(
            *operands,
            out_avals=tuple(out_avals),
            in_names=tuple(all_names),
            out_names=tuple(out_names),
            lowering_input_output_aliases=(),
            sim_require_finite=True,
            sim_require_nnan=True,
            nc=nc,
        )
        return tuple(outs)

    devices = jax.devices()[:N_CORES]
    mesh = Mesh(np.asarray(devices), ("core",))
    nspecs = n_params + len(out_names)
    fn = jax.jit(
        shard_map(
            _body,
            mesh=mesh,
            in_specs=(PartitionSpec("core"),) * nspecs,
            out_specs=(PartitionSpec("core"),) * len(out_names),
            check_rep=False,
        ),
        keep_unused=True,
    )
    sharding = NamedSharding(mesh, PartitionSpec("core"))
    return fn, in_names, zero_outs, sharding


_STAGED = {}


def _get_runner(dt_name, reps):
    key = (dt_name, reps)
    if key not in _RUNNER_CACHE:
        nc = _get_nc(dt_name, reps)
        _RUNNER_CACHE[key] = _make_runner(nc)
    return _RUNNER_CACHE[key]


def _stage(dt_name, in_maps, in_names, zero_outs, sharding, fresh=False):
    import jax

    if fresh or dt_name not in _STAGED:
        concat_in = [
            jax.device_put(
                np.concatenate([m[name] for m in in_maps], axis=0), sharding
            )
            for name in in_names
        ]
        concat_zero = [
            jax.device_put(
                np.zeros((N_CORES * z.shape[0], *z.shape[1:]), z.dtype), sharding
            )
            for z in zero_outs
        ]
        jax.block_until_ready(concat_in)
        _STAGED[dt_name] = (concat_in, concat_zero)
    return _STAGED[dt_name]


def _run_n(fn, concat_in, concat_zero, n):
    import time

    import jax

    t0 = time.perf_counter()
    last = None
    for _ in range(n):
        last = fn(*concat_in, *concat_zero)
    jax.block_until_ready(last)
    return time.perf_counter() - t0


def time_kernel_ns(n_iter=24, reps=9, rounds=5):
    """Differential HW time per kernel invocation, in ns.

    Times N pipelined executions of the reps=1 and reps=R NEFFs, interleaved
    (A/B alternating, min over rounds) so axon per-call dispatch drift
    (~4 ms/call, +-0.5 ms over minutes) cancels out of the slope."""
    import jax

    assert _LAST_IN_MAPS is not None, "call kernel() first"
    dt_name = os.environ.get("K_DTYPE", "float16")
    runners = {}
    for r in (1, reps):
        fn, in_names, zero_outs, sharding = _get_runner(dt_name, r)
        ci, cz = _stage(dt_name, _LAST_IN_MAPS, in_names, zero_outs, sharding)
        jax.block_until_ready(fn(*ci, *cz))  # compile + warm
        jax.block_until_ready(fn(*ci, *cz))
        runners[r] = (fn, ci, cz)
    t1 = tR = float("inf")
    for _ in range(rounds):
        t1 = min(t1, _run_n(*runners[1], n_iter))
        tR = min(tR, _run_n(*runners[reps], n_iter))
    per_rep = (tR - t1) / (n_iter * (reps - 1))
    if os.environ.get("K_DEBUG"):
        print(
            f"timing: T1={t1 / n_iter * 1e6:.1f} us/call, "
            f"T{reps}={tR / n_iter * 1e6:.1f} us/call, "
            f"diff/rep={per_rep * 1e6:.1f} us"
        )
    return per_rep * 1e9


# revision 14
# speedup vs baseline: 1.5918x; 1.0795x over previous
"""Local2d (locally-connected conv, unshared weights) Trainium2 kernel.

Problem: out[b,o,h,w] = sum_{i,k,l} weight[o,h,w,i,k,l] * xpad[b,i,h+k,w+l] + bias[o,h,w]
  x: [64, 64, 32, 32] f32, weight: [128, 32, 32, 64, 3, 3] f32, bias: [128, 32, 32] f32
  out: [64, 128, 32, 32] f32

Strategy: shard the 32 output rows h across 8 cores (4 rows each). Each output
location (h,w) is an independent GEMM: [o=128] x [ikl=576] @ [ikl=576] x [b=64].
Host pre-transposes weight to [h, ikl, w, o] and pre-extracts patches to
[h, ikl, w, b] (both fp16) so the contraction dim lands on SBUF partitions with
large contiguous DMAs and no on-chip transposes. Per location: 5 PSUM-accumulated
matmuls over ikl chunks (4x128 + 64), then a DVE bias-add copy into an output
tile [o, w, b] written back once per row. Output is reassembled/transposed on
host. Matmul inputs in fp16 (exact products, fp32 PSUM accumulation): rel err
vs the fp32 reference ~5e-4.
"""

import os
import numpy as np

B, C_IN, C_OUT, KS, H, W = 64, 64, 128, 3, 32, 32
H_OUT, W_OUT = 32, 32
N_CORES = 8
H_PER = H_OUT // N_CORES  # 4
IKL = C_IN * KS * KS  # 576
NCHUNK = 5
IKLP = NCHUNK * 128  # 640, ikl zero-padded so every chunk is K=128 (FWL-eligible)

_NC_CACHE = {}
_RUNNER_CACHE = {}
_LAST_IN_MAPS = None
LAST_RESULT = None


def _split_multiwaits(nc):
    """This container's walrus accepts at most ONE sync-wait per instruction.
    Hoist extra waits onto single-wait NoOps on the same engine, inserted
    immediately before (engine streams are in-order, sem waits are >=-monotonic,
    so this is semantics-preserving)."""
    import concourse.mybir as mybir

    ctr = 0
    hist = {}
    for f in nc.m.functions:
        for blk in f.blocks:
            insts = list(blk.instructions)
            changed = False
            newlist = []
            for inst in insts:
                si = inst.sync_info
                if si is not None and si.on_wait and len(si.on_wait) > 1:
                    tname = type(inst).__name__
                    hist[tname] = hist.get(tname, 0) + 1
                    waits = list(si.on_wait)
                    for wt in waits[:-1]:
                        nop = mybir.InstNoOp(name=f"splitwait-{ctr}", ins=[], outs=[])
                        ctr += 1
                        nop.engine = inst.engine
                        nop.sync_info = mybir.SyncInfo(on_wait=[wt], on_update=[])
                        newlist.append(nop)
                    inst.sync_info = mybir.SyncInfo(
                        on_wait=[waits[-1]], on_update=list(si.on_update or [])
                    )
                    changed = True
                newlist.append(inst)
            if changed:
                blk.instructions = newlist
    if os.environ.get("K_DEBUG"):
        print(f"split_multiwaits: {ctr} extra waits hoisted; by type: {hist}")
    return ctr


def _build_nc(dt_name, reps=1):
    import concourse.bass as bass
    import concourse.mybir as mybir
    import concourse.tile as tile

    dt_in = getattr(mybir.dt, dt_name)
    nc = bass.Bass()
    w_d = nc.dram_tensor("w", [H_PER, IKL, W_OUT, C_OUT], dt_in, kind="ExternalInput")
    p_d = nc.dram_tensor("p", [H_PER, IKL, W_OUT, B], dt_in, kind="ExternalInput")
    b_d = nc.dram_tensor(
        "bias", [C_OUT, H_PER, W_OUT], mybir.dt.float32, kind="ExternalInput"
    )
    o_d = nc.dram_tensor(
        "out", [C_OUT, H_PER, W_OUT, B], mybir.dt.float16, kind="ExternalOutput"
    )

    with tile.TileContext(nc) as tc:
        with (
            tc.tile_pool(name="wp", bufs=2) as wp,
            tc.tile_pool(name="pp", bufs=2) as pp,
            tc.tile_pool(name="op", bufs=2) as op,
            tc.tile_pool(name="bp", bufs=1) as bp,
            tc.tile_pool(name="psp", bufs=8, space="PSUM") as psp,
        ):
            bias_sb = bp.tile([C_OUT, H_PER, W_OUT], mybir.dt.float32, name="bias_sb")
            nc.gpsimd.dma_start(bias_sb[:], b_d[:])
            for rep in range(reps):
                for h in range(H_PER):
                    # alternate the two HWDGE rings between the big streams;
                    # chunks 0-3 (ikl 0:512) as one merged DMA in [p, c, w, x]
                    # layout, ragged chunk 4 (ikl 512:576, K=64) separately.
                    weng = nc.sync if h % 2 == 0 else nc.scalar
                    peng = nc.scalar if h % 2 == 0 else nc.sync
                    wm = wp.tile(
                        [128, 4, W_OUT, C_OUT], dt_in, tag="wm", name=f"wm_{rep}_{h}"
                    )
                    weng.dma_start(
                        wm[:], w_d[h, 0:512].rearrange("(c p) w o -> p c w o", p=128)
                    )
                    w4 = wp.tile(
                        [64, W_OUT, C_OUT], dt_in, tag="w4", name=f"w4_{rep}_{h}"
                    )
                    weng.dma_start(w4[:], w_d[h, 512:IKL])
                    pm = pp.tile(
                        [128, 4, W_OUT, B], dt_in, tag="pm", name=f"pm_{rep}_{h}"
                    )
                    peng.dma_start(
                        pm[:], p_d[h, 0:512].rearrange("(c p) w b -> p c w b", p=128)
                    )
                    p4 = pp.tile([64, W_OUT, B], dt_in, tag="p4", name=f"p4_{rep}_{h}")
                    peng.dma_start(p4[:], p_d[h, 512:IKL])
                    ot = op.tile(
                        [C_OUT, W_OUT, B], mybir.dt.float16, tag="ot",
                        name=f"ot_{rep}_{h}",
                    )
                    for w in range(W_OUT):
                        ps = psp.tile(
                            [C_OUT, B], mybir.dt.float32, tag="ps",
                            name=f"ps_{rep}_{h}_{w}",
                        )
                        for ci in range(4):
                            nc.tensor.matmul(
                                ps[:],
                                wm[:, ci, w, :],
                                pm[:, ci, w, :],
                                start=(ci == 0),
                                stop=False,
                            )
                        nc.tensor.matmul(
                            ps[:], w4[:, w, :], p4[:, w, :], start=False, stop=True
                        )
                        nc.vector.tensor_scalar_add(
                            ot[:, w, :], ps[:], bias_sb[:, h, w : w + 1]
                        )
                    nc.gpsimd.dma_start(o_d[:, h], ot[:])

    _split_multiwaits(nc)
    return nc


def _get_nc(dt_name, reps=1):
    key = (dt_name, reps)
    if key not in _NC_CACHE:
        _NC_CACHE[key] = _build_nc(dt_name, reps)
    return _NC_CACHE[key]


def _prepare_in_maps(x, weight, bias, dt_np):
    x = np.asarray(x, dtype=np.float32)
    weight = np.asarray(weight, dtype=np.float32)
    bias = np.asarray(bias, dtype=np.float32)

    # patches P[h, i*9+k*3+l, w, b] = xpad[b, i, h+k, w+l]
    xp = np.zeros((B, C_IN, H + 2, W + 2), dtype=dt_np)
    xp[:, :, 1 : H + 1, 1 : W + 1] = x
    s = xp.strides
    v = np.lib.stride_tricks.as_strided(
        xp, (B, C_IN, KS, KS, H_OUT, W_OUT), (s[0], s[1], s[2], s[3], s[2], s[3])
    )
    P = v.transpose(4, 1, 2, 3, 5, 0).reshape(H_OUT, IKL, W_OUT, B)

    # weight -> [h, ikl, w, o]
    Wt = weight.reshape(C_OUT, H_OUT, W_OUT, IKL).transpose(1, 3, 2, 0).astype(dt_np)

    in_maps = []
    for c in range(N_CORES):
        h0 = c * H_PER
        in_maps.append(
            {
                "w": np.ascontiguousarray(Wt[h0 : h0 + H_PER]),
                "p": np.ascontiguousarray(P[h0 : h0 + H_PER]),
                "bias": np.ascontiguousarray(bias[:, h0 : h0 + H_PER, :]),
            }
        )
    return in_maps


def kernel(x, weight, bias):
    global _LAST_IN_MAPS

    dt_name = os.environ.get("K_DTYPE", "float16")
    dt_np = {"float16": np.float16, "float32": np.float32}[dt_name]

    in_maps = _prepare_in_maps(x, weight, bias, dt_np)
    _LAST_IN_MAPS = in_maps

    fn, in_names, zero_outs, sharding = _get_runner(dt_name, 1)
    concat_in, concat_zero = _stage(
        dt_name, in_maps, in_names, zero_outs, sharding, fresh=True
    )
    outs = fn(*concat_in, *concat_zero)
    out_global = np.asarray(outs[0])  # (8*128, H_PER, 32, 64) fp16

    out = np.concatenate(
        [out_global[c * C_OUT : (c + 1) * C_OUT] for c in range(N_CORES)], axis=1
    )  # [o, 32, 32, b]
    return np.ascontiguousarray(
        out.transpose(3, 0, 1, 2).astype(np.float32)
    )


# ---------------------------------------------------------------------------
# Timing (NTFF profiling is unavailable in this container: antenv.axon_hooks
# missing). Measure differentially instead: jit the NEFF exec for reps=1 and
# reps=R bodies, pre-stage inputs on devices, time N pipelined executions of
# each, and report (T_R - T_1) / (N * (R - 1)).
# ---------------------------------------------------------------------------


def _make_runner(nc):
    import jax
    import concourse.mybir as mybir
    from concourse.bass2jax import (
        _bass_exec_p,
        install_neuronx_cc_hook,
        partition_id_tensor,
    )
    from jax.experimental.shard_map import shard_map
    from jax.sharding import Mesh, NamedSharding, PartitionSpec

    install_neuronx_cc_hook()

    partition_name = nc.partition_id_tensor.name if nc.partition_id_tensor else None
    in_names, out_names, out_avals, zero_outs = [], [], [], []
    for alloc in nc.m.functions[0].allocations:
        if not isinstance(alloc, mybir.MemoryLocationSet):
            continue
        name = alloc.memorylocations[0].name
        if alloc.kind == "ExternalInput":
            if name != partition_name:
                in_names.append(name)
        elif alloc.kind == "ExternalOutput":
            out_names.append(name)
            shape = tuple(alloc.tensor_shape)
            dtype = mybir.dt.np(alloc.dtype)
            out_avals.append(jax.core.ShapedArray(shape, dtype))
            zero_outs.append(np.zeros(shape, dtype))
    n_params = len(in_names)
    all_names = in_names + out_names
    if partition_name is not None:
        all_names = all_names + [partition_name]

    def _body(*args):
        operands = list(args)
        if partition_name is not None:
            operands.append(partition_id_tensor())
        outs = _bass_exec_p.bind(
            *operands,
            out_avals=tuple(out_avals),
            in_names=tuple(all_names),
            out_names=tuple(out_names),
            lowering_input_output_aliases=(),
            sim_require_finite=True,
            sim_require_nnan=True,
            nc=nc,
        )
        return tuple(outs)

    devices = jax.devices()[:N_CORES]
    mesh = Mesh(np.asarray(devices), ("core",))
    nspecs = n_params + len(out_names)
    fn = jax.jit(
        shard_map(
            _body,
            mesh=mesh,
            in_specs=(PartitionSpec("core"),) * nspecs,
            out_specs=(PartitionSpec("core"),) * len(out_names),
            check_rep=False,
        ),
        keep_unused=True,
    )
    sharding = NamedSharding(mesh, PartitionSpec("core"))
    return fn, in_names, zero_outs, sharding


_STAGED = {}


def _get_runner(dt_name, reps):
    key = (dt_name, reps)
    if key not in _RUNNER_CACHE:
        nc = _get_nc(dt_name, reps)
        _RUNNER_CACHE[key] = _make_runner(nc)
    return _RUNNER_CACHE[key]


def _stage(dt_name, in_maps, in_names, zero_outs, sharding, fresh=False):
    import jax

    if fresh or dt_name not in _STAGED:
        concat_in = [
            jax.device_put(
                np.concatenate([m[name] for m in in_maps], axis=0), sharding
            )
            for name in in_names
        ]
        concat_zero = [
            jax.device_put(
                np.zeros((N_CORES * z.shape[0], *z.shape[1:]), z.dtype), sharding
            )
            for z in zero_outs
        ]
        jax.block_until_ready(concat_in)
        _STAGED[dt_name] = (concat_in, concat_zero)
    return _STAGED[dt_name]


def _run_n(fn, concat_in, concat_zero, n):
    import time

    import jax

    t0 = time.perf_counter()
    last = None
    for _ in range(n):
        last = fn(*concat_in, *concat_zero)
    jax.block_until_ready(last)
    return time.perf_counter() - t0


def time_kernel_ns(n_iter=24, reps=9, rounds=5):
    """Differential HW time per kernel invocation, in ns.

    Times N pipelined executions of the reps=1 and reps=R NEFFs, interleaved
    (A/B alternating, min over rounds) so axon per-call dispatch drift
    (~4 ms/call, +-0.5 ms over minutes) cancels out of the slope."""
    import jax

    assert _LAST_IN_MAPS is not None, "call kernel() first"
    dt_name = os.environ.get("K_DTYPE", "float16")
    runners = {}
    for r in (1, reps):
        fn, in_names, zero_outs, sharding = _get_runner(dt_name, r)
        ci, cz = _stage(dt_name, _LAST_IN_MAPS, in_names, zero_outs, sharding)
        jax.block_until_ready(fn(*ci, *cz))  # compile + warm
        jax.block_until_ready(fn(*ci, *cz))
        runners[r] = (fn, ci, cz)
    t1 = tR = float("inf")
    for _ in range(rounds):
        t1 = min(t1, _run_n(*runners[1], n_iter))
        tR = min(tR, _run_n(*runners[reps], n_iter))
    per_rep = (tR - t1) / (n_iter * (reps - 1))
    if os.environ.get("K_DEBUG"):
        print(
            f"timing: T1={t1 / n_iter * 1e6:.1f} us/call, "
            f"T{reps}={tR / n_iter * 1e6:.1f} us/call, "
            f"diff/rep={per_rep * 1e6:.1f} us"
        )
    return per_rep * 1e9


# revision 20
# speedup vs baseline: 2.1155x; 1.3290x over previous
"""Local2d (locally-connected conv, unshared weights) Trainium2 kernel.

Problem: out[b,o,h,w] = sum_{i,k,l} weight[o,h,w,i,k,l] * xpad[b,i,h+k,w+l] + bias[o,h,w]
  x: [64, 64, 32, 32] f32, weight: [128, 32, 32, 64, 3, 3] f32, bias: [128, 32, 32] f32
  out: [64, 128, 32, 32] f32

Strategy: shard the 32 output rows h across 8 cores (4 rows each). Each output
location (h,w) is an independent GEMM: [o=128] x [ikl=576] @ [ikl=576] x [b=64].
Host pre-transposes weight to [h, ikl, w, o] and pre-extracts patches to
[h, ikl, w, b] (both fp16) so the contraction dim lands on SBUF partitions with
large contiguous DMAs and no on-chip transposes. Per location: 5 PSUM-accumulated
matmuls over ikl chunks (4x128 + 64), then a DVE bias-add copy into an output
tile [o, w, b] written back once per row. Output is reassembled/transposed on
host. Matmul inputs in fp16 (exact products, fp32 PSUM accumulation): rel err
vs the fp32 reference ~5e-4.
"""

import os
import numpy as np

B, C_IN, C_OUT, KS, H, W = 64, 64, 128, 3, 32, 32
H_OUT, W_OUT = 32, 32
N_CORES = 8
H_PER = H_OUT // N_CORES  # 4
IKL = C_IN * KS * KS  # 576
NCHUNK = 5
IKLP = NCHUNK * 128  # 640, ikl zero-padded so every chunk is K=128 (FWL-eligible)

_NC_CACHE = {}
_RUNNER_CACHE = {}
_LAST_IN_MAPS = None
LAST_RESULT = None


def _split_multiwaits(nc):
    """This container's walrus accepts at most ONE sync-wait per instruction.
    Hoist extra waits onto single-wait NoOps on the same engine, inserted
    immediately before (engine streams are in-order, sem waits are >=-monotonic,
    so this is semantics-preserving)."""
    import concourse.mybir as mybir

    ctr = 0
    hist = {}
    for f in nc.m.functions:
        for blk in f.blocks:
            insts = list(blk.instructions)
            changed = False
            newlist = []
            for inst in insts:
                si = inst.sync_info
                if si is not None and si.on_wait and len(si.on_wait) > 1:
                    tname = type(inst).__name__
                    hist[tname] = hist.get(tname, 0) + 1
                    waits = list(si.on_wait)
                    for wt in waits[:-1]:
                        nop = mybir.InstNoOp(name=f"splitwait-{ctr}", ins=[], outs=[])
                        ctr += 1
                        nop.engine = inst.engine
                        nop.sync_info = mybir.SyncInfo(on_wait=[wt], on_update=[])
                        newlist.append(nop)
                    inst.sync_info = mybir.SyncInfo(
                        on_wait=[waits[-1]], on_update=list(si.on_update or [])
                    )
                    changed = True
                newlist.append(inst)
            if changed:
                blk.instructions = newlist
    if os.environ.get("K_DEBUG"):
        print(f"split_multiwaits: {ctr} extra waits hoisted; by type: {hist}")
    return ctr


def _build_nc(dt_name, reps=1):
    import concourse.bass as bass
    import concourse.mybir as mybir
    import concourse.tile as tile

    dt_in = getattr(mybir.dt, dt_name)
    nc = bass.Bass()
    w_d = nc.dram_tensor("w", [H_PER, IKL, W_OUT, C_OUT], dt_in, kind="ExternalInput")
    p_d = nc.dram_tensor("p", [H_PER, IKL, W_OUT, B], dt_in, kind="ExternalInput")
    b_d = nc.dram_tensor(
        "bias", [C_OUT, H_PER, W_OUT], mybir.dt.float32, kind="ExternalInput"
    )
    o_d = nc.dram_tensor(
        "out", [C_OUT, H_PER, W_OUT, B], mybir.dt.float16, kind="ExternalOutput"
    )

    with tile.TileContext(nc) as tc:
        with (
            tc.tile_pool(name="wp", bufs=2) as wp,
            tc.tile_pool(name="pp", bufs=2) as pp,
            tc.tile_pool(name="op", bufs=2) as op,
            tc.tile_pool(name="bp", bufs=1) as bp,
            tc.tile_pool(name="psp", bufs=8, space="PSUM") as psp,
        ):
            bias_sb = bp.tile([C_OUT, H_PER, W_OUT], mybir.dt.float32, name="bias_sb")
            nc.gpsimd.dma_start(bias_sb[:], b_d[:])
            for rep in range(reps):
                for h in range(H_PER):
                    # alternate the two HWDGE rings between the big streams;
                    # chunks 0-3 (ikl 0:512) as one merged DMA in [p, c, w, x]
                    # layout, ragged chunk 4 (ikl 512:576, K=64) separately.
                    weng = nc.sync if h % 2 == 0 else nc.scalar
                    peng = nc.scalar if h % 2 == 0 else nc.sync
                    wm = wp.tile(
                        [128, 4, W_OUT, C_OUT], dt_in, tag="wm", name=f"wm_{rep}_{h}"
                    )
                    weng.dma_start(
                        wm[:], w_d[h, 0:512].rearrange("(c p) w o -> p c w o", p=128)
                    )
                    w4 = wp.tile(
                        [64, W_OUT, C_OUT], dt_in, tag="w4", name=f"w4_{rep}_{h}"
                    )
                    weng.dma_start(w4[:], w_d[h, 512:IKL])
                    pm = pp.tile(
                        [128, 4, W_OUT, B], dt_in, tag="pm", name=f"pm_{rep}_{h}"
                    )
                    peng.dma_start(
                        pm[:], p_d[h, 0:512].rearrange("(c p) w b -> p c w b", p=128)
                    )
                    p4 = pp.tile([64, W_OUT, B], dt_in, tag="p4", name=f"p4_{rep}_{h}")
                    peng.dma_start(p4[:], p_d[h, 512:IKL])
                    ot = op.tile(
                        [C_OUT, W_OUT, B], mybir.dt.float16, tag="ot",
                        name=f"ot_{rep}_{h}",
                    )
                    for w in range(W_OUT):
                        ps = psp.tile(
                            [C_OUT, B], mybir.dt.float32, tag="ps",
                            name=f"ps_{rep}_{h}_{w}",
                        )
                        for ci in range(4):
                            nc.tensor.matmul(
                                ps[:],
                                wm[:, ci, w, :],
                                pm[:, ci, w, :],
                                start=(ci == 0),
                                stop=False,
                            )
                        nc.tensor.matmul(
                            ps[:], w4[:, w, :], p4[:, w, :], start=False, stop=True
                        )
                        nc.vector.tensor_scalar_add(
                            ot[:, w, :], ps[:], bias_sb[:, h, w : w + 1]
                        )
                    nc.gpsimd.dma_start(o_d[:, h], ot[:])

    _split_multiwaits(nc)
    return nc


def _get_nc(dt_name, reps=1):
    key = (dt_name, reps)
    if key not in _NC_CACHE:
        _NC_CACHE[key] = _build_nc(dt_name, reps)
    return _NC_CACHE[key]


def _prepare_in_maps(x, weight, bias, dt_np):
    x = np.asarray(x, dtype=np.float32)
    weight = np.asarray(weight, dtype=np.float32)
    bias = np.asarray(bias, dtype=np.float32)

    # patches P[h, i*9+k*3+l, w, b] = xpad[b, i, h+k, w+l]
    xp = np.zeros((B, C_IN, H + 2, W + 2), dtype=dt_np)
    xp[:, :, 1 : H + 1, 1 : W + 1] = x
    s = xp.strides
    v = np.lib.stride_tricks.as_strided(
        xp, (B, C_IN, KS, KS, H_OUT, W_OUT), (s[0], s[1], s[2], s[3], s[2], s[3])
    )
    P = v.transpose(4, 1, 2, 3, 5, 0).reshape(H_OUT, IKL, W_OUT, B)

    # weight -> [h, ikl, w, o]
    Wt = weight.reshape(C_OUT, H_OUT, W_OUT, IKL).transpose(1, 3, 2, 0).astype(dt_np)

    in_maps = []
    for c in range(N_CORES):
        h0 = c * H_PER
        in_maps.append(
            {
                "w": np.ascontiguousarray(Wt[h0 : h0 + H_PER]),
                "p": np.ascontiguousarray(P[h0 : h0 + H_PER]),
                "bias": np.ascontiguousarray(bias[:, h0 : h0 + H_PER, :]),
            }
        )
    return in_maps


def kernel(x, weight, bias):
    global _LAST_IN_MAPS

    dt_name = os.environ.get("K_DTYPE", "float16")
    dt_np = {"float16": np.float16, "float32": np.float32}[dt_name]

    in_maps = _prepare_in_maps(x, weight, bias, dt_np)
    _LAST_IN_MAPS = in_maps

    fn, in_names, zero_outs, sharding = _get_runner(dt_name, 1)
    concat_in, concat_zero = _stage(
        dt_name, in_maps, in_names, zero_outs, sharding, fresh=True
    )
    outs = fn(*concat_in, *concat_zero)
    out_global = np.asarray(outs[0])  # (8*128, H_PER, 32, 64) fp16

    out = np.concatenate(
        [out_global[c * C_OUT : (c + 1) * C_OUT] for c in range(N_CORES)], axis=1
    )  # [o, 32, 32, b]
    return np.ascontiguousarray(
        out.transpose(3, 0, 1, 2).astype(np.float32)
    )


# ---------------------------------------------------------------------------
# Timing (NTFF profiling is unavailable in this container: antenv.axon_hooks
# missing). Measure differentially instead: jit the NEFF exec for reps=1 and
# reps=R bodies, pre-stage inputs on devices, time N pipelined executions of
# each, and report (T_R - T_1) / (N * (R - 1)).
# ---------------------------------------------------------------------------


def _make_runner(nc):
    import jax
    import concourse.mybir as mybir
    from concourse.bass2jax import (
        _bass_exec_p,
        install_neuronx_cc_hook,
        partition_id_tensor,
    )
    from jax.experimental.shard_map import shard_map
    from jax.sharding import Mesh, NamedSharding, PartitionSpec

    install_neuronx_cc_hook()

    partition_name = nc.partition_id_tensor.name if nc.partition_id_tensor else None
    in_names, out_names, out_avals, zero_outs = [], [], [], []
    for alloc in nc.m.functions[0].allocations:
        if not isinstance(alloc, mybir.MemoryLocationSet):
            continue
        name = alloc.memorylocations[0].name
        if alloc.kind == "ExternalInput":
            if name != partition_name:
                in_names.append(name)
        elif alloc.kind == "ExternalOutput":
            out_names.append(name)
            shape = tuple(alloc.tensor_shape)
            dtype = mybir.dt.np(alloc.dtype)
            out_avals.append(jax.core.ShapedArray(shape, dtype))
            zero_outs.append(np.zeros(shape, dtype))
    n_params = len(in_names)
    all_names = in_names + out_names
    if partition_name is not None:
        all_names = all_names + [partition_name]

    def _body(*args):
        operands = list(args)
        if partition_name is not None:
            operands.append(partition_id_tensor())
        outs = _bass_exec_p.bind(
            *operands,
            out_avals=tuple(out_avals),
            in_names=tuple(all_names),
            out_names=tuple(out_names),
            lowering_input_output_aliases=(),
            sim_require_finite=True,
            sim_require_nnan=True,
            nc=nc,
        )
        return tuple(outs)

    devices = jax.devices()[:N_CORES]
    mesh = Mesh(np.asarray(devices), ("core",))
    nspecs = n_params + len(out_names)
    fn = jax.jit(
        shard_map(
            _body,
            mesh=mesh,
            in_specs=(PartitionSpec("core"),) * nspecs,
            out_specs=(PartitionSpec("core"),) * len(out_names),
            check_rep=False,
        ),
        keep_unused=True,
    )
    sharding = NamedSharding(mesh, PartitionSpec("core"))
    return fn, in_names, zero_outs, sharding


_STAGED = {}


def _get_runner(dt_name, reps):
    key = (dt_name, reps)
    if key not in _RUNNER_CACHE:
        nc = _get_nc(dt_name, reps)
        _RUNNER_CACHE[key] = _make_runner(nc)
    return _RUNNER_CACHE[key]


def _stage(dt_name, in_maps, in_names, zero_outs, sharding, fresh=False):
    import jax

    if fresh or dt_name not in _STAGED:
        concat_in = [
            jax.device_put(
                np.concatenate([m[name] for m in in_maps], axis=0), sharding
            )
            for name in in_names
        ]
        concat_zero = [
            jax.device_put(
                np.zeros((N_CORES * z.shape[0], *z.shape[1:]), z.dtype), sharding
            )
            for z in zero_outs
        ]
        jax.block_until_ready(concat_in)
        _STAGED[dt_name] = (concat_in, concat_zero)
    return _STAGED[dt_name]


def _run_n(fn, concat_in, concat_zero, n):
    import time

    import jax

    t0 = time.perf_counter()
    last = None
    for _ in range(n):
        last = fn(*concat_in, *concat_zero)
    jax.block_until_ready(last)
    return time.perf_counter() - t0


def time_kernel_ns(n_iter=24, reps=9, rounds=5):
    """Differential HW time per kernel invocation, in ns.

    Times N pipelined executions of the reps=1 and reps=R NEFFs, interleaved
    (A/B alternating, min over rounds) so axon per-call dispatch drift
    (~4 ms/call, +-0.5 ms over minutes) cancels out of the slope."""
    import jax

    assert _LAST_IN_MAPS is not None, "call kernel() first"
    dt_name = os.environ.get("K_DTYPE", "float16")
    runners = {}
    for r in (1, reps):
        fn, in_names, zero_outs, sharding = _get_runner(dt_name, r)
        ci, cz = _stage(dt_name, _LAST_IN_MAPS, in_names, zero_outs, sharding)
        jax.block_until_ready(fn(*ci, *cz))  # compile + warm
        jax.block_until_ready(fn(*ci, *cz))
        runners[r] = (fn, ci, cz)
    t1 = tR = float("inf")
    for _ in range(rounds):
        t1 = min(t1, _run_n(*runners[1], n_iter))
        tR = min(tR, _run_n(*runners[reps], n_iter))
    per_rep = (tR - t1) / (n_iter * (reps - 1))
    if os.environ.get("K_DEBUG"):
        print(
            f"timing: T1={t1 / n_iter * 1e6:.1f} us/call, "
            f"T{reps}={tR / n_iter * 1e6:.1f} us/call, "
            f"diff/rep={per_rep * 1e6:.1f} us"
        )
    return per_rep * 1e9
